# revision 1
# baseline (speedup 1.0000x reference)
"""AttentionSubsample kernel for 8 trn2 NeuronCores.

Sharding: head-parallel (8 heads -> 8 cores), each core handles its head for
both batches through attn@v + hardswish; final projection is sharded by
output channels after an AllGather of the per-head attention outputs.

Key tricks:
- All matmuls in bf16 with fp32 PSUM accumulation.
- S^T layout for the attention matrix (k on partitions, q on free) so both
  QK^T and attn@V are natural matmuls (no transposes of big tensors).
- The relative-position bias is factored out of the softmax numerator:
  exp(qk*scale + b) = exp(qk*scale) * exp(b); exp(b) is a small host-side
  table, expanded to a full (padded-k, q) bf16 tensor per head, streamed from
  HBM and multiplied in on the vector engine (2x bf16 mode). Padded k rows
  get exp(b)=0 which also kills them in the softmax denominator.
- Softmax denominator comes free from the attn@V matmul by appending a ones
  column to V (33rd stationary column).
- BatchNorms (training mode) computed on-device via bn_stats/bn_aggr over the
  full (B*N) token range; affine folded into a per-partition tensor_scalar.
"""

import numpy as np
import ml_dtypes

import concourse.bass as bass
import concourse.mybir as mybir
import concourse.tile as tile
from concourse import bacc
from contextlib import ExitStack
from concourse.bass_utils import run_bass_kernel_spmd

BF16 = mybir.dt.bfloat16
F32 = mybir.dt.float32
bf16 = ml_dtypes.bfloat16

B = 2
ROW, COL = 63, 84
ROW_, COL_ = 32, 42
N = ROW * COL            # 5292 kv tokens
NQ = ROW_ * COL_         # 1344 q tokens
NPAD = 5376              # 42*128 padded kv tokens
KT = NPAD // 128         # 42 k-tiles
QC = 448                 # q chunk
NQC = NQ // QC           # 3
CIN = 256
H = 8
KD = 16
DV = 32
HKV = KD + DV            # 48 per-head kv channels
KVP = 64                 # padded kv rows: k at 0:16, v at 32:64 (32-aligned)
OC = 64                  # per-core slice of the 512 output channels
GRP = 3                  # k-tiles per exp group
NGRP = KT // GRP         # 14
EPS = 1e-5
SCALE = KD ** -0.5
NCORES = 8

LAST_EXEC_NS = None
_prog_cache = {}


def _build_program(debug=False):
    nc = bacc.Bacc(num_devices=NCORES)

    xT = nc.dram_tensor("xT", [B, 2, 128, NPAD], BF16, kind="ExternalInput")
    xsT = nc.dram_tensor("xsT", [B, 2, 128, NQ], BF16, kind="ExternalInput")
    wkvT = nc.dram_tensor("wkvT", [2, 128, KVP], BF16, kind="ExternalInput")
    wqT = nc.dram_tensor("wqT", [2, 128, KD], BF16, kind="ExternalInput")
    wpT = nc.dram_tensor("wpT", [2, 128, OC], BF16, kind="ExternalInput")
    kv_gb = nc.dram_tensor("kv_gb", [KVP, 2], F32, kind="ExternalInput")
    q_gb = nc.dram_tensor("q_gb", [KD, 2], F32, kind="ExternalInput")
    p_gb = nc.dram_tensor("p_gb", [OC, 2], F32, kind="ExternalInput")
    ebT = nc.dram_tensor("ebT", [NQC, NGRP, 128, GRP * QC], BF16,
                         kind="ExternalInput")
    yT = nc.dram_tensor("yT", [OC, B * NQ], F32, kind="ExternalOutput")
    if debug:
        dbg = {
            "dbg_ykv": nc.dram_tensor("dbg_ykv", [KVP, B, NPAD], BF16,
                                      kind="ExternalOutput"),
            "dbg_kT": nc.dram_tensor("dbg_kT", [KD, B, NPAD], BF16,
                                     kind="ExternalOutput"),
            "dbg_qT": nc.dram_tensor("dbg_qT", [KD, B, NQ], BF16,
                                     kind="ExternalOutput"),
            "dbg_vaug": nc.dram_tensor("dbg_vaug", [128, B, KT, DV + 1], BF16,
                                       kind="ExternalOutput"),
            "dbg_hsT": nc.dram_tensor("dbg_hsT", [DV, B, NQ], BF16,
                                      kind="ExternalOutput"),
            "dbg_yp": nc.dram_tensor("dbg_yp", [OC, B * NQ], F32,
                                     kind="ExternalOutput"),
            "dbg_mvkv": nc.dram_tensor("dbg_mvkv", [KVP, 2], F32,
                                       kind="ExternalOutput"),
            "dbg_sp0": nc.dram_tensor("dbg_sp0", [128, GRP, QC], BF16,
                                      kind="ExternalOutput"),
            "dbg_vtd": nc.dram_tensor("dbg_vtd", [128, B, KT, DV], BF16,
                                      kind="ExternalOutput"),
        }

    with ExitStack() as ctx:
        tc = ctx.enter_context(tile.TileContext(nc))
        const = ctx.enter_context(tc.tile_pool(name="const", bufs=1))
        big = ctx.enter_context(tc.tile_pool(name="big", bufs=1))
        vtp = ctx.enter_context(tc.tile_pool(name="vtp", bufs=1))
        spool = ctx.enter_context(tc.tile_pool(name="spool", bufs=6))
        ebpool = ctx.enter_context(tc.tile_pool(name="ebpool", bufs=6))
        small = ctx.enter_context(tc.tile_pool(name="small", bufs=4))
        drain = ctx.enter_context(tc.tile_pool(name="drain", bufs=3))
        psA = ctx.enter_context(tc.tile_pool(name="psA", bufs=2, space="PSUM"))
        psB = ctx.enter_context(tc.tile_pool(name="psB", bufs=2, space="PSUM"))
        dram = ctx.enter_context(tc.tile_pool(name="dram", bufs=4, space="DRAM"))

        mult = mybir.AluOpType.mult
        add = mybir.AluOpType.add
        amin = mybir.AluOpType.min
        Act = mybir.ActivationFunctionType

        # ------------------------- load inputs -------------------------
        xt_sb = big.tile([128, B, 2, NPAD], BF16, tag="xt")
        xs_sb = big.tile([128, B, 2, NQ], BF16, tag="xs")
        for b in range(B):
            for c in range(2):
                nc.sync.dma_start(out=xt_sb[:, b, c, :], in_=xT[b, c])
            for c in range(2):
                nc.sync.dma_start(out=xs_sb[:, b, c, :], in_=xsT[b, c])
        wkv_sb = const.tile([128, 2, KVP], BF16, tag="wkv")
        wq_sb = const.tile([128, 2, KD], BF16, tag="wq")
        wp_sb = const.tile([128, 2, OC], BF16, tag="wp")
        for c in range(2):
            nc.sync.dma_start(out=wkv_sb[:, c, :], in_=wkvT[c])
            nc.sync.dma_start(out=wq_sb[:, c, :], in_=wqT[c])
            nc.sync.dma_start(out=wp_sb[:, c, :], in_=wpT[c])
        kvgb_sb = const.tile([KVP, 2], F32, tag="kvgb")
        qgb_sb = const.tile([KD, 2], F32, tag="qgb")
        pgb_sb = const.tile([OC, 2], F32, tag="pgb")
        nc.sync.dma_start(out=kvgb_sb, in_=kv_gb[:, :])
        nc.sync.dma_start(out=qgb_sb, in_=q_gb[:, :])
        nc.sync.dma_start(out=pgb_sb, in_=p_gb[:, :])
        eps_t = const.tile([128, 1], F32, tag="eps")
        nc.vector.memset(eps_t, EPS)
        three_t = const.tile([128, 1], F32, tag="three")
        nc.vector.memset(three_t, 3.0)
        ones1_t = const.tile([1, DV], F32, tag="ones1")
        nc.vector.memset(ones1_t, 1.0)

        # ------------------------- projections -------------------------
        TCH = 448
        NT_KV = NPAD // TCH   # 12
        y_kv = big.tile([KVP, B, NPAD], BF16, tag="ykv")
        for b in range(B):
            for t in range(NT_KV):
                ps = psB.tile([KVP, TCH], F32, tag="ps_small")
                for c in range(2):
                    nc.tensor.matmul(ps, wkv_sb[:, c, :],
                                     xt_sb[:, b, c, bass.ts(t, TCH)],
                                     start=(c == 0), stop=(c == 1))
                nc.scalar.copy(out=y_kv[:, b, bass.ts(t, TCH)], in_=ps)

        y_q = big.tile([KD, B, NQ], BF16, tag="yq")
        for b in range(B):
            for t in range(NQC):
                ps = psB.tile([KD, TCH], F32, tag="ps_small")
                for c in range(2):
                    nc.tensor.matmul(ps, wq_sb[:, c, :],
                                     xs_sb[:, b, c, bass.ts(t, QC)],
                                     start=(c == 0), stop=(c == 1))
                nc.scalar.copy(out=y_q[:, b, bass.ts(t, QC)], in_=ps)

        # ------------------------- batch norms -------------------------
        def bn_scale_shift(mv, gb, P, name):
            # returns s, t with s = g/sqrt(var+eps), t = beta - mu*s
            s = small.tile([P, 1], F32, tag=f"s_{name}")
            t = small.tile([P, 1], F32, tag=f"t_{name}")
            nc.scalar.activation(out=s, in_=mv[:, 1:2], func=Act.Sqrt,
                                 bias=eps_t[0:P])
            nc.vector.reciprocal(out=s, in_=s)
            nc.vector.tensor_mul(s, s, gb[:, 0:1])
            nc.vector.tensor_mul(t, mv[:, 0:1], s)
            nc.vector.tensor_scalar(out=t, in0=t, scalar1=-1.0, scalar2=None,
                                    op0=mult)
            nc.vector.tensor_add(t, t, gb[:, 1:2])
            return s, t

        # kv stats over the real (un-padded) token range: 5292 = 12*441
        st_kv = small.tile([KVP, 2 * 12, 6], F32, tag="st_kv")
        for b in range(B):
            for i in range(12):
                nc.vector.bn_stats(out=st_kv[:, b * 12 + i, :],
                                   in_=y_kv[:, b, bass.ds(i * 441, 441)])
        mv_kv = small.tile([KVP, 2], F32, tag="mv_kv")
        nc.vector.bn_aggr(out=mv_kv, in_=st_kv)
        s_kv, t_kv = bn_scale_shift(mv_kv, kvgb_sb, KVP, "kv")

        st_q = small.tile([KD, 2 * NQC, 6], F32, tag="st_q")
        for b in range(B):
            for i in range(NQC):
                nc.vector.bn_stats(out=st_q[:, b * NQC + i, :],
                                   in_=y_q[:, b, bass.ts(i, QC)])
        mv_q = small.tile([KD, 2], F32, tag="mv_q")
        nc.vector.bn_aggr(out=mv_q, in_=st_q)
        s_q, t_q = bn_scale_shift(mv_q, qgb_sb, KD, "q")

        # normalized k^T, q^T (channel-major, replicated at partition rows
        # 0/32/64 for 3-way row-group-packed QK matmuls) and v (token-major
        # + ones column)
        kT = big.tile([KD, B, NPAD], BF16, tag="kT")
        qT = big.tile([KD, B, NQ], BF16, tag="qT")
        v_aug = big.tile([128, B, KT, DV + 1], BF16, tag="vaug")
        for b in range(B):
            nc.vector.tensor_scalar(out=kT[0:KD, b, :], in0=y_kv[0:KD, b, :],
                                    scalar1=s_kv[0:KD], scalar2=t_kv[0:KD],
                                    op0=mult, op1=add)
            nc.vector.tensor_scalar(out=qT[0:KD, b, :], in0=y_q[:, b, :],
                                    scalar1=s_q, scalar2=t_q,
                                    op0=mult, op1=add)
        for b in range(B):
            vTn = vtp.tile([DV, NPAD], BF16, tag="vTn")
            nc.vector.tensor_scalar(out=vTn, in0=y_kv[32:KVP, b, :],
                                    scalar1=s_kv[32:KVP], scalar2=t_kv[32:KVP],
                                    op0=mult, op1=add)
            vtd = vtp.tile([128, KT, DV], BF16, tag="vtd")
            nc.sync.dma_start_transpose(out=vtd, in_=vTn)
            if debug:
                nc.sync.dma_start(out=dbg["dbg_vtd"][:, b, :, :], in_=vtd)
            nc.vector.tensor_copy(v_aug[:, b, :, 0:DV], vtd)
            nc.vector.memset(v_aug[:, b, :, DV:DV + 1], 1.0)

        if debug:
            nc.sync.dma_start(out=dbg["dbg_ykv"][:, :, :], in_=y_kv)
            nc.sync.dma_start(out=dbg["dbg_kT"][:, :, :], in_=kT)
            nc.sync.dma_start(out=dbg["dbg_qT"][:, :, :], in_=qT)
            nc.sync.dma_start(out=dbg["dbg_vaug"][:, :, :, :], in_=v_aug)
            nc.sync.dma_start(out=dbg["dbg_mvkv"][:, :], in_=mv_kv)

        # ------------------------- attention -------------------------
        # qc-outer so each exp(bias) tile is DMA'd once and shared by both
        # batches; per-chunk AllGather is launched as soon as a chunk drains
        # so the collective hides under the remaining attention work.
        hsT = big.tile([DV, B, NQ], BF16, tag="xs")
        hs_bounce = dram.tile([NQC, DV, B * QC], BF16, tag="hs_bounce")
        hs_all = dram.tile([NQC, H * DV, B * QC], BF16, tag="hs_all")
        for qc in range(NQC):
            avs = []
            for b in range(B):
                av_t = psB.tile([DV + 1, QC], F32, tag="ps_small")
                avs.append(av_t)
            for g in range(NGRP):
                eb = ebpool.tile([128, GRP, QC], BF16, tag="eb")
                nc.sync.dma_start(
                    out=eb,
                    in_=ebT[qc, g].rearrange("p (i q) -> p i q", i=GRP))
                for b in range(B):
                    qk = psA.tile([128, GRP, 512], F32, tag="qk")
                    for i in range(GRP):
                        j = g * GRP + i
                        nc.tensor.matmul(qk[:, i, 0:QC],
                                         kT[:, b, bass.ts(j, 128)],
                                         qT[:, b, bass.ts(qc, QC)],
                                         start=True, stop=True)
                    sp = spool.tile([128, GRP, QC], BF16, tag="sp")
                    nc.scalar.activation(out=sp, in_=qk[:, :, 0:QC],
                                         func=Act.Exp, scale=SCALE)
                    nc.vector.tensor_mul(sp, sp, eb)
                    if debug and b == 0 and qc == 0 and g == 0:
                        nc.sync.dma_start(out=dbg["dbg_sp0"][:, :, :], in_=sp)
                    for i in range(GRP):
                        j = g * GRP + i
                        nc.tensor.matmul(avs[b], v_aug[:, b, j, :],
                                         sp[:, i, :],
                                         start=(j == 0), stop=(j == KT - 1),
                                         skip_group_check=True)
            for b in range(B):
                # park the accumulator in SBUF right away so the PSUM slot
                # frees for the next chunk; drain math runs DVE-only so the
                # ACT exp pipeline never blocks behind it
                av_sb = drain.tile([DV + 1, QC], F32, tag="av_sb")
                nc.vector.tensor_copy(av_sb, avs[b])
                av = av_sb
                rec = drain.tile([1, QC], F32, tag="rec")
                nc.vector.reciprocal(out=rec, in_=av[DV:DV + 1, :])
                # broadcast 1/denominator across the 32 value rows via PE
                recb = psB.tile([DV, QC], F32, tag="ps_small")
                nc.tensor.matmul(recb, ones1_t, rec, start=True, stop=True)
                xo = drain.tile([DV, QC], F32, tag="xo")
                nc.vector.tensor_mul(xo, av[0:DV, :], recb)
                r3 = drain.tile([DV, QC], F32, tag="r3")
                nc.vector.tensor_scalar(out=r3, in0=xo, scalar1=3.0,
                                        scalar2=0.0, op0=add,
                                        op1=mybir.AluOpType.max)
                nc.vector.tensor_scalar(out=r3, in0=r3, scalar1=6.0,
                                        scalar2=1.0 / 6.0, op0=amin, op1=mult)
                nc.vector.tensor_mul(hsT[:, b, bass.ts(qc, QC)], xo, r3)
            nc.sync.dma_start(
                out=hs_bounce[qc].rearrange("d (b q) -> d b q", b=B),
                in_=hsT[:, :, bass.ts(qc, QC)])
            nc.gpsimd.collective_compute(
                "AllGather", mybir.AluOpType.bypass,
                replica_groups=[list(range(NCORES))],
                ins=[hs_bounce[qc].opt()],
                outs=[hs_all[qc].opt()])

        if debug:
            nc.sync.dma_start(out=dbg["dbg_hsT"][:, :, :], in_=hsT)

        # --------------------- projection (chunked) ---------------------
        y_p = big.tile([OC, B * NQ], F32, tag="yq")
        for qc in range(NQC):
            hsall_sb = drain.tile([128, 2, B * QC], BF16, tag="hsall")
            for c in range(2):
                nc.sync.dma_start(out=hsall_sb[:, c, :],
                                  in_=hs_all[qc, bass.ts(c, 128), :])
            for b in range(B):
                ps = psB.tile([OC, QC], F32, tag="ps_small")
                for c in range(2):
                    nc.tensor.matmul(ps, wp_sb[:, c, :],
                                     hsall_sb[:, c, bass.ds(b * QC, QC)],
                                     start=(c == 0), stop=(c == 1))
                nc.scalar.copy(out=y_p[:, bass.ds(b * NQ + qc * QC, QC)],
                               in_=ps)
        if debug:
            nc.sync.dma_start(out=dbg["dbg_yp"][:, :], in_=y_p)
        st_p = small.tile([OC, B * NQ // QC, 6], F32, tag="st_p")
        for i in range(B * NQ // QC):
            nc.vector.bn_stats(out=st_p[:, i, :], in_=y_p[:, bass.ts(i, QC)])
        mv_p = small.tile([OC, 2], F32, tag="mv_p")
        nc.vector.bn_aggr(out=mv_p, in_=st_p)
        s_p, t_p = bn_scale_shift(mv_p, pgb_sb, OC, "p")
        nc.vector.tensor_scalar(out=y_p, in0=y_p, scalar1=s_p, scalar2=t_p,
                                op0=mult, op1=add)
        nc.sync.dma_start(out=yT[:, :], in_=y_p)

    nc.finalize()
    return nc


def _prep_inputs(x, kv_w, kv_g, kv_b, q_w, q_g, q_b, proj_w, proj_g, proj_b,
                 bias_table, bias_idxs):
    """Host-side sharding/layout prep. Returns list of 8 per-core input maps."""
    x = np.asarray(x, np.float32)
    # x^T padded: (B, 2, 128, NPAD)
    xt = np.zeros((B, 2, 128, NPAD), np.float32)
    xTt = x.transpose(0, 2, 1)  # (B, 256, N)
    xt[:, :, :, :N] = xTt.reshape(B, 2, 128, N)
    xt = xt.astype(bf16)
    xs = x.reshape(B, ROW, COL, CIN)[:, ::2, ::2].reshape(B, NQ, CIN)
    xst = xs.transpose(0, 2, 1).reshape(B, 2, 128, NQ).astype(bf16)

    # exp(bias) tables per head, padded-k zeroed, laid out (NQC, NGRP, 128, GRP*QC)
    rank2 = np.asarray(bias_idxs)[0].reshape(ROW, COL)  # (dr, dc) -> id
    table2 = np.asarray(bias_table, np.float32)[:, rank2]  # (H, 63, 84)
    eb2 = np.exp(table2)
    kk = np.arange(N)
    qq = np.arange(NQ)
    DRm = np.abs(kk[:, None] // COL - 2 * (qq[None, :] // COL_))
    DCm = np.abs(kk[:, None] % COL - 2 * (qq[None, :] % COL_))

    in_maps = []
    for h in range(H):
        ebf = np.zeros((NPAD, NQ), np.float32)
        ebf[:N] = eb2[h][DRm, DCm]
        # (NPAD, NQ) -> (NQC, NGRP, 128, GRP*QC)
        ebl = (ebf.reshape(NGRP, GRP, 128, NQC, QC)
               .transpose(3, 0, 2, 1, 4)
               .reshape(NQC, NGRP, 128, GRP * QC)).astype(bf16)
        sl = slice(h * HKV, (h + 1) * HKV)
        slq = slice(h * KD, (h + 1) * KD)
        slo = slice(h * OC, (h + 1) * OC)
        # kv weights/gains padded to 64 rows: k at 0:16, v at 32:64
        wkv_pad = np.zeros((KVP, CIN), np.float32)
        wkv_pad[0:KD] = np.asarray(kv_w, np.float32)[sl][0:KD]
        wkv_pad[32:KVP] = np.asarray(kv_w, np.float32)[sl][KD:HKV]
        kvgb_pad = np.zeros((KVP, 2), np.float32)
        kvgb_pad[:, 0] = 1.0
        kvgb_pad[0:KD, 0] = np.asarray(kv_g, np.float32)[sl][0:KD]
        kvgb_pad[0:KD, 1] = np.asarray(kv_b, np.float32)[sl][0:KD]
        kvgb_pad[32:KVP, 0] = np.asarray(kv_g, np.float32)[sl][KD:HKV]
        kvgb_pad[32:KVP, 1] = np.asarray(kv_b, np.float32)[sl][KD:HKV]
        in_maps.append({
            "xT": xt,
            "xsT": xst,
            "wkvT": np.ascontiguousarray(
                wkv_pad.T.reshape(2, 128, KVP)).astype(bf16),
            "wqT": np.ascontiguousarray(
                np.asarray(q_w, np.float32)[slq].T.reshape(2, 128, KD)
            ).astype(bf16),
            "wpT": np.ascontiguousarray(
                np.asarray(proj_w, np.float32)[slo].T.reshape(2, 128, OC)
            ).astype(bf16),
            "kv_gb": np.ascontiguousarray(kvgb_pad),
            "q_gb": np.ascontiguousarray(np.stack(
                [np.asarray(q_g, np.float32)[slq],
                 np.asarray(q_b, np.float32)[slq]], axis=1)),
            "p_gb": np.ascontiguousarray(np.stack(
                [np.asarray(proj_g, np.float32)[slo],
                 np.asarray(proj_b, np.float32)[slo]], axis=1)),
            "ebT": ebl,
        })
    return in_maps


def kernel(x, kv_w, kv_g, kv_b, q_w, q_g, q_b, proj_w, proj_g, proj_b,
           bias_table, bias_idxs, _trace=False):
    global LAST_EXEC_NS
    if "nc" not in _prog_cache:
        _prog_cache["nc"] = _build_program()
    nc = _prog_cache["nc"]
    in_maps = _prep_inputs(x, kv_w, kv_g, kv_b, q_w, q_g, q_b,
                           proj_w, proj_g, proj_b, bias_table, bias_idxs)
    res = run_bass_kernel_spmd(nc, in_maps, core_ids=list(range(NCORES)),
                               trace=_trace)
    LAST_EXEC_NS = res.exec_time_ns
    yts = [np.asarray(r["yT"]) for r in res.results]  # each (OC, B*NQ)
    y = np.concatenate(yts, axis=0)                   # (512, B*NQ)
    return np.ascontiguousarray(
        y.T.reshape(B, NQ, H * OC).astype(np.float32))

